# revision 1
# baseline (speedup 1.0000x reference)
"""BERT self-attention layer (B=8, S=1024, H=12, Dh=64) on 8 trn2 NeuronCores.

Sharding: pure data-parallel over batch (1 batch item per core, weights
replicated).

Matmul inputs use float32r (single-pass fp32 matmuls, 4x the throughput of
the fp32 hi/lo-split path, ~14-bit multiply mantissa).  The residual + LN
path stays exact fp32, so the final error is ~1e-5 relative.

Transposes (x -> xT, W -> WT) avoid the PE entirely: a 32x32-block-permuted
DMA load + DVE StreamTranspose (in-block 32x32 transpose) + ScalarE cast to
f32r.

Per-core dataflow (T = "transposed layout", features on partitions):
  xT   = dve-transpose(x)                                 [6][128k, 1024s]
  QT   = WqT.T-chain @ xT   (lhsT = WqT tiles)            [6][128d, 1024s]
  KT   likewise                                           [6][128d, 1024s]
  V    = xT.T-chain @ WvT   (normal layout, per-head 65-  [8][128s, 780]
         wide blocks: 64 value cols + a ones col)
  per head pair (A, B), per ks-tile j:   (A/B use PE row groups 0/64)
    sT[j]   = KT[h].T-slice @ QT[h]    (psum [128ks, 1024q])
    eT[j]   = exp(sT[j]/8 + mask)      (ScalarE, psum -> sbuf, f32r)
    ctx_ps += Vaug[j,h].T @ eT[j]      (psum [65, 1024q]; row 64 = denom)
  rec  = 1/ctx_ps[64] (custom-DVE recip) -> gpsimd partition_broadcast
  ctxT[h] = ctx_ps[0:64] * rec                            [6][128d, 1024q]
  out[st] = LN(x[st] + ctxT.T-chain @ WdT)  fused via STT/accum_out
"""

import os
import numpy as np
from contextlib import ExitStack

import concourse.bass as bass
import concourse.bacc as bacc
import concourse.tile as tile
from concourse import mybir
from concourse._compat import with_exitstack
from concourse.bass import ts, ds
from concourse.bass_utils import run_bass_kernel_spmd
import concourse.bass_utils as _bu
from concourse.masks import make_identity

H = 12
DH = 64
D = 768
S = 1024
P = 128
KT_ = D // P  # 6 feature tiles
ST_ = S // P  # 8 sequence tiles
HB = DH + 1  # per-head V block width (64 value cols + ones col)
EPS = 1e-12
F32 = mybir.dt.float32
F32R = mybir.dt.float32r
FT = mybir.ActivationFunctionType
ALU = mybir.AluOpType
N_CORES = 8
MDT = F32R  # dtype of tiles feeding matmuls

# Let walrus dedupe back-to-back LDWEIGHTS of the same stationary operand
# (disabled by default in bir_verify_and_optimise; toggled via env).
if os.environ.get("BERT_LDW_OPT", "1") == "1" and not getattr(
    _bu, "_ldw_opt_patched", False
):
    _orig_run_command = _bu.run_command

    def _run_command_ldw(cmd, *a, **kw):
        cmd = [
            "--enable-ldw-opt=true" if c == "--enable-ldw-opt=false" else c
            for c in cmd
        ]
        return _orig_run_command(cmd, *a, **kw)

    _bu.run_command = _run_command_ldw
    _bu._ldw_opt_patched = True


def _permuted_src(ap, col0, n_free_blocks):
    """DRAM AP enumerating src[32J+r, col0+c] for r,c in 32x32 blocks, in
    (r, J, c) order — the 32x32-block-permuted load feeding StreamTranspose."""
    rs = ap.ap[0][0]
    return bass.AP(
        tensor=ap.tensor,
        offset=ap.offset + col0,
        ap=[[rs, 32], [32 * rs, n_free_blocks], [1, 32]],
    )


def _load_T(nc, dest_pool, scratch_pool, src_ap, n_k_tiles, n_free, tag):
    """Transpose a [n_free, 128*n_k_tiles] DRAM matrix into n_k_tiles f32r
    sbuf tiles [128, n_free] without touching the PE:
    permuted DMA -> DVE 32x32 StreamTranspose -> ScalarE cast to f32r.
    DMA issues alternate between the two HWDGE engines (SP, ACT) so the
    issue cost doesn't serialize on one queue."""
    out = []
    dma_engines = [nc.sync, nc.scalar]
    for kt in range(n_k_tiles):
        perm = scratch_pool.tile([P, n_free], F32, tag="tsp", bufs=2,
                                 name=f"{tag}p{kt}", padded_shape=[P, S])
        p4 = perm.rearrange("(i r) (j c) -> i r j c", r=32, c=32)
        for i in range(4):
            dma_engines[(kt * 4 + i) % 2].dma_start(
                out=p4[i],
                in_=_permuted_src(src_ap, 128 * kt + 32 * i, n_free // 32),
            )
        tf = scratch_pool.tile([P, n_free], F32, tag="tst", bufs=2,
                               name=f"{tag}t{kt}", padded_shape=[P, S])
        nc.vector.transpose(tf, perm)
        wt = dest_pool.tile([P, n_free], MDT, tag=tag, bufs=n_k_tiles,
                            name=f"{tag}_{src_ap.tensor.name}_{kt}")
        nc.scalar.copy(wt, tf)
        out.append(wt)
    return out


@with_exitstack
def bert_attn_kernel(
    ctx: ExitStack,
    tc: tile.TileContext,
    out_ap: bass.AP,
    x_ap: bass.AP,
    mask_ap: bass.AP,
    wq_ap: bass.AP,
    bq_ap: bass.AP,
    wk_ap: bass.AP,
    bk_ap: bass.AP,
    wv_ap: bass.AP,
    bv_ap: bass.AP,
    wd_ap: bass.AP,
    bd_ap: bass.AP,
    g_ap: bass.AP,
    b_ap: bass.AP,
    use_mask: bool,
    use_qkv_bias: bool,
    use_dense_bias: bool,
    use_ln_affine: bool,
):
    nc = tc.nc

    # ---- persistent pools ----
    const_pool = ctx.enter_context(tc.tile_pool(name="const", bufs=1))
    qkv_pool = ctx.enter_context(tc.tile_pool(name="qkv", bufs=1))
    ctxT_pool = ctx.enter_context(tc.tile_pool(name="ctxT", bufs=1))

    eps_t = const_pool.tile([P, 1], F32)
    nc.vector.memset(eps_t, EPS)
    ident = const_pool.tile([P, P], F32)
    make_identity(nc, ident)

    maskT = None
    if use_mask:
        maskT = const_pool.tile([P, ST_], F32)
        nc.sync.dma_start(out=maskT, in_=mask_ap.rearrange("(t p) -> p t", p=P))

    bq_t = bk_t = bv_bc = None
    if use_qkv_bias:
        bq_t = const_pool.tile([P, KT_], F32)
        nc.sync.dma_start(out=bq_t, in_=bq_ap.rearrange("(t p) -> p t", p=P))
        bk_t = const_pool.tile([P, KT_], F32)
        nc.sync.dma_start(out=bk_t, in_=bk_ap.rearrange("(t p) -> p t", p=P))
        bv_bc = const_pool.tile([P, D], F32)
        _bcast_load(nc, bv_bc, bv_ap, P)
    ones1 = bd_row = None
    if use_dense_bias:
        ones1 = const_pool.tile([1, P], MDT)
        nc.vector.memset(ones1.bitcast(mybir.dt.uint32), 0x3F800000)
        bdf = const_pool.tile([1, D], F32)
        nc.sync.dma_start(out=bdf, in_=bd_ap[None, :])
        bd_row = const_pool.tile([1, D], MDT)
        nc.scalar.copy(bd_row, bdf)
    g_bc = b_bc = None
    if use_ln_affine:
        g_bc = const_pool.tile([P, D], F32)
        _bcast_load(nc, g_bc, g_ap, P)
        b_bc = const_pool.tile([P, D], F32)
        _bcast_load(nc, b_bc, b_ap, P)

    QT = [qkv_pool.tile([P, S], MDT, tag="QT", bufs=KT_, name=f"QT{i}")
          for i in range(KT_)]
    KTt = [qkv_pool.tile([P, S], MDT, tag="KTt", bufs=KT_, name=f"KTt{i}")
           for i in range(KT_)]
    vaug = [qkv_pool.tile([P, H * HB], MDT, tag="vaug", bufs=ST_,
                          name=f"vaug{i}") for i in range(ST_)]
    xn = [qkv_pool.tile([P, D], F32, tag="xn", bufs=ST_, name=f"xn{i}")
          for i in range(ST_)]

    # =========== phase 1: transposes + QKV projections ===========
    with tc.tile_pool(name="tsc", bufs=1) as tsc_pool, \
         tc.tile_pool(name="wT", bufs=1) as wT_pool, \
         tc.tile_pool(name="xt", bufs=1) as xt_pool, \
         tc.tile_pool(name="ps_t", bufs=4, space="PSUM") as psum_t, \
         tc.tile_pool(name="ps_qk", bufs=2, space="PSUM") as psum_qk:

        # x: fast row-major load + PE transpose (the PE is otherwise idle
        # here and this overlaps with the permuted-DMA weight loads)
        for st in range(ST_):
            nc.sync.dma_start(out=xn[st], in_=x_ap[ts(st, P), :])
        xT = [xt_pool.tile([P, S], MDT, tag="xT", bufs=KT_, name=f"xT{i}")
              for i in range(KT_)]
        for st in range(ST_):
            for kt in range(KT_):
                tps = psum_t.tile([P, P], F32)
                nc.tensor.transpose(tps, xn[st][:, ts(kt, P)], ident)
                nc.scalar.copy(xT[kt][:, ts(st, P)], tps)

        # V = x @ Wv.T in natural layout, per-head [v0..v63, 1] blocks;
        # ones pre-filled, value cols overwritten by the evac.
        for v in vaug:
            nc.vector.memset(v.bitcast(mybir.dt.uint32), 0x3F800000)
        wvT = _load_T(nc, wT_pool, tsc_pool, wv_ap, KT_, D, "wvT")
        v3 = [v.rearrange("p (h c) -> p h c", c=HB) for v in vaug]
        for st in range(ST_):
            vps = psum_qk.tile([P, S], F32, tag="mmps", bufs=2,
                               name="vps")[:, 0:D]
            for kt in range(KT_):
                for c0 in range(0, D, 512):
                    cw = min(512, D - c0)
                    nc.tensor.matmul(
                        vps[:, ds(c0, cw)],
                        lhsT=xT[kt][:, ts(st, P)],
                        rhs=wvT[kt][:, ds(c0, cw)],
                        start=(kt == 0),
                        stop=(kt == KT_ - 1),
                    )
            vps3 = vps.rearrange("p (h c) -> p h c", c=DH)
            if use_qkv_bias:
                bv3 = bv_bc.rearrange("p (h c) -> p h c", c=DH)
                nc.vector.tensor_add(v3[st][:, :, 0:DH], vps3, bv3)
            else:
                nc.vector.tensor_copy(v3[st][:, :, 0:DH], vps3)

        for w_ap_i, bias_t, dest, wtag in (
            (wq_ap, bq_t, QT, "wqkT"),
            (wk_ap, bk_t, KTt, "wqkT"),
        ):
            wqT = _load_T(nc, wT_pool, tsc_pool, w_ap_i, KT_, D, wtag)
            for mt in range(KT_):
                qps = psum_qk.tile([P, S], F32, tag="mmps", bufs=2, name="qps")
                for kt in range(KT_):
                    for qc in range(0, S, 512):
                        nc.tensor.matmul(
                            qps[:, ds(qc, 512)],
                            lhsT=wqT[kt][:, ts(mt, P)],
                            rhs=xT[kt][:, ds(qc, 512)],
                            start=(kt == 0),
                            stop=(kt == KT_ - 1),
                        )
                if use_qkv_bias:
                    nc.vector.tensor_scalar_add(dest[mt], qps,
                                                bias_t[:, mt : mt + 1])
                else:
                    nc.vector.tensor_copy(dest[mt], qps)

    ctxT = [ctxT_pool.tile([P, S], MDT, tag="ctxT", bufs=KT_, name=f"ctxT{i}")
            for i in range(KT_)]
    wdT_pool = ctx.enter_context(tc.tile_pool(name="wdTp", bufs=1))
    tsc2_pool = ctx.enter_context(tc.tile_pool(name="tsc2", bufs=1))
    wdT = None

    # =========== phase 2: attention, two heads (PE row groups) at a time ====
    with tc.tile_pool(name="expT", bufs=1) as exp_pool, \
         tc.tile_pool(name="den", bufs=1) as den_pool, \
         tc.tile_pool(name="ps_s", bufs=2, space="PSUM") as psum_s, \
         tc.tile_pool(name="ps_ctx", bufs=2, space="PSUM") as psum_ctx:

        for pr in range(H // 2):
            if pr == 1:
                # overlap the Wd transpose-load with attention compute
                wdT = _load_T(nc, wdT_pool, tsc2_pool, wd_ap, KT_, D, "wdT")
            cc = []
            for half in range(2):
                cc.append(psum_ctx.tile([HB, S], F32, tag="cps", bufs=2,
                                        name=f"cps{half}"))
            for j in range(ST_):
                ee = []
                for half in range(2):
                    h = 2 * pr + half
                    hp = DH * half
                    sps = psum_s.tile([P, S], F32, tag="sps", bufs=2,
                                      name=f"sps{half}")
                    for qc in range(0, S, 512):
                        nc.tensor.matmul(
                            sps[:, ds(qc, 512)],
                            lhsT=KTt[pr][hp : hp + DH, ts(j, P)],
                            rhs=QT[pr][hp : hp + DH, ds(qc, 512)],
                            start=True,
                            stop=True,
                        )
                    e = exp_pool.tile([P, S], MDT, tag="e", bufs=4,
                                      name=f"e{half}")
                    nc.scalar.activation(
                        e, sps, FT.Exp,
                        bias=(maskT[:, j : j + 1] if use_mask else 0.0),
                        scale=0.125,
                    )
                    ee.append(e)
                for half in range(2):
                    h = 2 * pr + half
                    for qc in range(0, S, 512):
                        nc.tensor.matmul(
                            cc[half][:, ds(qc, 512)],
                            lhsT=vaug[j][:, ds(HB * h, HB)],
                            rhs=ee[half][:, ds(qc, 512)],
                            start=(j == 0),
                            stop=(j == ST_ - 1),
                        )
            for half in range(2):
                hp = DH * half
                den_sb = den_pool.tile([1, S], F32, tag="den_sb", bufs=2)
                nc.vector.tensor_copy(den_sb, cc[half][DH : DH + 1, :])
                rec = den_pool.tile([1, S], F32, tag="rec", bufs=2)
                nc.vector.reciprocal_approx_fast(rec, den_sb)
                recb = den_pool.tile([DH, S], F32, tag="recb", bufs=2)
                nc.gpsimd.partition_broadcast(recb, rec)
                nc.vector.tensor_mul(ctxT[pr][hp : hp + DH, :],
                                     cc[half][0:DH, :], recb)

    # =========== phase 3: dense + residual + layernorm ===========
    with tc.tile_pool(name="ln", bufs=2) as ln_pool, \
         tc.tile_pool(name="stat", bufs=4) as stat_pool, \
         tc.tile_pool(name="osb", bufs=3) as out_pool, \
         tc.tile_pool(name="ps_o", bufs=2, space="PSUM") as psum_o:

        for st in range(ST_):
            xr = xn[st]
            ops = psum_o.tile([P, D], F32, tag="ops", bufs=2)
            if use_dense_bias:
                for c0 in range(0, D, 512):
                    cw = min(512, D - c0)
                    nc.tensor.matmul(
                        ops[:, ds(c0, cw)], lhsT=ones1,
                        rhs=bd_row[:, ds(c0, cw)], start=True, stop=False,
                    )
            for kt in range(KT_):
                for c0 in range(0, D, 512):
                    cw = min(512, D - c0)
                    nc.tensor.matmul(
                        ops[:, ds(c0, cw)],
                        lhsT=ctxT[kt][:, ts(st, P)],
                        rhs=wdT[kt][:, ds(c0, cw)],
                        start=(kt == 0 and not use_dense_bias),
                        stop=(kt == KT_ - 1),
                    )
            # full = dense_out + x, accumulating the row-sum on the fly
            full = ln_pool.tile([P, D], F32, tag="full")
            sums = stat_pool.tile([P, 1], F32, tag="sums")
            nc.vector.scalar_tensor_tensor(
                out=full, in0=ops, scalar=1.0, in1=xr,
                op0=ALU.mult, op1=ALU.add, accum_out=sums,
            )
            # sum of squares on ScalarE; sq is a dead store
            sq = ln_pool.tile([P, D], F32, tag="sq")
            ssq = stat_pool.tile([P, 1], F32, tag="ssq")
            nc.scalar.activation(sq, full, FT.Square, accum_out=ssq)
            mu = stat_pool.tile([P, 1], F32, tag="mu")
            nc.vector.tensor_scalar_mul(mu, sums, 1.0 / D)
            mu2 = stat_pool.tile([P, 1], F32, tag="mu2")
            nc.vector.tensor_scalar_mul(mu2, mu, mu)
            var = stat_pool.tile([P, 1], F32, tag="var")
            nc.vector.scalar_tensor_tensor(
                out=var, in0=ssq, scalar=1.0 / D, in1=mu2,
                op0=ALU.mult, op1=ALU.subtract,
            )
            std = stat_pool.tile([P, 1], F32, tag="std")
            nc.scalar.activation(std, var, FT.Sqrt, bias=eps_t)
            rstd = stat_pool.tile([P, 1], F32, tag="rstd")
            nc.vector.reciprocal(rstd, std)
            osb = out_pool.tile([P, D], F32, tag="osb")
            nc.vector.tensor_scalar(
                out=osb, in0=full, scalar1=mu, scalar2=rstd,
                op0=ALU.subtract, op1=ALU.mult,
            )
            if use_ln_affine:
                nc.vector.tensor_mul(osb, osb, g_bc)
                nc.vector.tensor_add(osb, osb, b_bc)
            nc.sync.dma_start(out=out_ap[ts(st, P), :], in_=osb)


def _bcast_load(nc, out_tile, vec_ap, n_part):
    """DMA a [N] DRAM vector replicated across n_part partitions."""
    src = bass.AP(
        tensor=vec_ap.tensor,
        offset=vec_ap.offset,
        ap=[[0, n_part]] + [list(d) for d in vec_ap.ap],
    )
    nc.gpsimd.dma_start(out=out_tile, in_=src)


def build(flags):
    nc = bacc.Bacc(
        "TRN2", target_bir_lowering=False, debug=False, num_devices=N_CORES
    )
    aps = {}
    for name, shape in (
        ("hidden_states", [S, D]),
        ("attention_mask", [S]),
        ("Wq", [D, D]), ("bq", [D]),
        ("Wk", [D, D]), ("bk", [D]),
        ("Wv", [D, D]), ("bv", [D]),
        ("Wd", [D, D]), ("bd", [D]),
        ("ln_g", [D]), ("ln_b", [D]),
    ):
        aps[name] = nc.dram_tensor(name, shape, F32, kind="ExternalInput").ap()
    out = nc.dram_tensor("out", [S, D], F32, kind="ExternalOutput").ap()

    with tile.TileContext(nc) as tc:
        bert_attn_kernel(
            tc, out,
            aps["hidden_states"], aps["attention_mask"],
            aps["Wq"], aps["bq"], aps["Wk"], aps["bk"],
            aps["Wv"], aps["bv"], aps["Wd"], aps["bd"],
            aps["ln_g"], aps["ln_b"],
            *flags,
        )
    nc.compile()
    return nc


_CACHE = {}
last_results = None  # BassKernelResults of the most recent run (for test.py)


def kernel(**inputs):
    xs = {k: np.ascontiguousarray(np.asarray(v, dtype=np.float32))
          for k, v in inputs.items()}
    B = xs["hidden_states"].shape[0]
    assert B == N_CORES

    flags = (
        bool(np.any(xs["attention_mask"])),
        bool(np.any(xs["bq"]) or np.any(xs["bk"]) or np.any(xs["bv"])),
        bool(np.any(xs["bd"])),
        bool(np.any(xs["ln_g"] != 1.0) or np.any(xs["ln_b"])),
    )
    if flags not in _CACHE:
        _CACHE[flags] = build(flags)
    nc = _CACHE[flags]

    shared = {k: xs[k] for k in
              ("Wq", "bq", "Wk", "bk", "Wv", "bv", "Wd", "bd", "ln_g", "ln_b")}
    in_maps = [
        dict(
            hidden_states=xs["hidden_states"][i],
            attention_mask=np.ascontiguousarray(
                xs["attention_mask"][i].reshape(S)),
            **shared,
        )
        for i in range(N_CORES)
    ]
    trace = bool(int(os.environ.get("BERT_KERNEL_TRACE", "0")))
    res = run_bass_kernel_spmd(
        nc, in_maps, core_ids=list(range(N_CORES)), trace=trace
    )
    global last_results
    last_results = res
    return np.stack([res.results[i]["out"] for i in range(N_CORES)], axis=0)


if __name__ == "__main__":
    rng = np.random.default_rng(0)
    ins = {
        "hidden_states": rng.standard_normal((8, S, D), dtype=np.float32),
        "attention_mask": np.zeros((8, 1, 1, S), np.float32),
        "Wq": rng.standard_normal((D, D), dtype=np.float32) * 0.02,
        "bq": np.zeros(D, np.float32),
        "Wk": rng.standard_normal((D, D), dtype=np.float32) * 0.02,
        "bk": np.zeros(D, np.float32),
        "Wv": rng.standard_normal((D, D), dtype=np.float32) * 0.02,
        "bv": np.zeros(D, np.float32),
        "Wd": rng.standard_normal((D, D), dtype=np.float32) * 0.02,
        "bd": np.zeros(D, np.float32),
        "ln_g": np.ones(D, np.float32),
        "ln_b": np.zeros(D, np.float32),
    }
    out = kernel(**ins)
    print(out.shape, out.dtype, np.abs(out).max())

